# revision 25
# baseline (speedup 1.0000x reference)
"""Multi-head attention (B=4, N=2048, C=1024, H=16, D=64) on 8 TRN2 NeuronCores.

Sharding: core c owns (batch b = c//2, sequence half = c%2) -> 1024 query
tokens, all 16 heads.  Each core computes K/V for its OWN sequence half only;
the partner core's half arrives via a pairwise AllGather (replica groups
[2b, 2b+1]).  The gathered K/V use rank-order for the m axis on both cores,
which keeps K and V consistent (softmax is permutation-invariant in m).
Output is purely row-sharded -> host gather is a concat.

Device-side layout tricks (all transposes are done on the host):
- xT_aug  [1025, 1024] bf16: channel-major own-half x with a ones row.
- wqkvT_aug [1025, 3072] bf16: w_qkv^T; bias row drives the V bias via the
  ones row; Q/K biases are fused into the PSUM->SBUF copies per-partition.
- Scores are computed transposed (S^T[m, n]); softmax denominators come from
  a ones-column appended to V inside the PV matmul; normalization runs from
  an SBUF staging copy so PSUM banks recycle fast (keeps TensorE from ever
  idling >3.4us, which would trip the HAM clock gate).
- All matmuls in bf16 (f32 PSUM accumulate).
"""

import numpy as np
import ml_dtypes

import concourse.bass as bass
import concourse.mybir as mybir
import concourse.tile as tile
from concourse import bacc
from concourse.bass_utils import run_bass_kernel_spmd

B, N, C = 4, 2048, 1024
H, D = 16, 64
SCALE = D ** -0.5
NCORES = 8
NQ = N // 2          # query tokens per core (own half)
M = N                # key/value tokens after gather
CT = [128] * 8 + [1]

BF16 = mybir.dt.bfloat16
F32 = mybir.dt.float32

_CACHE = {}
LAST_RESULTS = None


def _build():
    nc = bacc.Bacc(
        "TRN2",
        target_bir_lowering=False,
        debug=False,
        enable_asserts=False,
        num_devices=NCORES,
    )
    xT = nc.dram_tensor("xT", [1025, M], BF16, kind="ExternalInput")
    xoT = nc.dram_tensor("xoT", [C, NQ], BF16, kind="ExternalInput")
    wqkvT = nc.dram_tensor("wqkvT", [1025, 3 * C], BF16, kind="ExternalInput")
    bqk = nc.dram_tensor("bqk", [2 * C, 1], F32, kind="ExternalInput")
    wprojT = nc.dram_tensor("wprojT", [C, C], BF16, kind="ExternalInput")
    bproj = nc.dram_tensor("bproj", [C, 1], F32, kind="ExternalInput")
    yT = nc.dram_tensor("yT", [C, NQ], F32, kind="ExternalOutput")

    groups = [[2 * b, 2 * b + 1] for b in range(B)]

    with tile.TileContext(nc) as tc:
        with (
            tc.tile_pool(name="persist", bufs=1) as pp,
            tc.tile_pool(name="psum", bufs=1, space="PSUM") as psp,
            tc.tile_pool(name="dram", bufs=1, space="DRAM") as dp,
        ):
            lp = tc.alloc_tile_pool(name="qkv_in", bufs=1)
            x_sb = []
            xo_sb = []
            wq_sb = []
            for ct in range(9):
                p = CT[ct]
                x_sb.append(lp.tile([p, M], BF16, tag=f"x{ct}", name=f"x{ct}"))
                wq_sb.append(lp.tile([p, 3 * C], BF16, tag=f"wq{ct}", name=f"wq{ct}"))
                if ct < 8:
                    xo_sb.append(lp.tile([128, NQ], BF16, tag=f"xo{ct}", name=f"xo{ct}"))
            # K-phase inputs first, then V-phase, then Q columns
            for ct in range(8):
                nc.sync.dma_start(xo_sb[ct][:, :], xoT[ct * 128 : (ct + 1) * 128, :])
                nc.sync.dma_start(
                    wq_sb[ct][:, C : 2 * C], wqkvT[ct * 128 : (ct + 1) * 128, C : 2 * C]
                )
            for ct in range(9):
                p = CT[ct]
                nc.sync.dma_start(x_sb[ct][:, :], xT[ct * 128 : ct * 128 + p, :])
                nc.sync.dma_start(
                    wq_sb[ct][:, 2 * C :], wqkvT[ct * 128 : ct * 128 + p, 2 * C :]
                )
            for ct in range(8):
                nc.sync.dma_start(
                    wq_sb[ct][:, 0:C], wqkvT[ct * 128 : (ct + 1) * 128, 0:C]
                )
            nc.sync.dma_start(wq_sb[8][:, 0 : 2 * C], wqkvT[1024:1025, 0 : 2 * C])
            bp_sb = []
            bq_sb = []
            bk_sb = []
            for i in range(8):
                t = pp.tile([128, 1], F32, tag=f"bp{i}", name=f"bp{i}")
                nc.sync.dma_start(t[:, :], bproj[i * 128 : (i + 1) * 128, :])
                bp_sb.append(t)
                t = pp.tile([128, 1], F32, tag=f"bq{i}", name=f"bq{i}")
                nc.sync.dma_start(t[:, :], bqk[i * 128 : (i + 1) * 128, :])
                bq_sb.append(t)
                t = pp.tile([128, 1], F32, tag=f"bk{i}", name=f"bk{i}")
                nc.sync.dma_start(t[:, :], bqk[C + i * 128 : C + (i + 1) * 128, :])
                bk_sb.append(t)

            QT_sb = [pp.tile([128, NQ], BF16, tag=f"qt{i}", name=f"qt{i}") for i in range(8)]
            KT_sb = [pp.tile([128, M], BF16, tag=f"kt{i}", name=f"kt{i}") for i in range(8)]
            V_sb = [pp.tile([128, H, D + 1], BF16, tag=f"v{mt}", name=f"v{mt}") for mt in range(16)]
            A_sb = [pp.tile([128, NQ], BF16, tag=f"a{i}", name=f"a{i}") for i in range(8)]

            # DRAM bounce buffers for the pairwise K/V AllGather (2 chunks each)
            k_in = [dp.tile([512, NQ], BF16, tag=f"ki{c}", name=f"ki{c}") for c in range(2)]
            k_out = [
                dp.tile([2, 512, NQ], BF16, tag=f"ko{c}", name=f"ko{c}")
                for c in range(2)
            ]
            # ---- K own-half first (from own-half x; bias fused in copy);
            # each 4-tile chunk's AllGather is issued as soon as it's staged.
            for i in range(8):
                c = i // 4
                kh = lp.tile([128, NQ], BF16, tag="kh", bufs=2, name="kh")
                ps = psp.tile([128, NQ], F32, tag="mm", bufs=2, name="psk")
                for ct in range(8):
                    for nch in range(2):
                        nc.tensor.matmul(
                            ps[:, nch * 512 : (nch + 1) * 512],
                            wq_sb[ct][:, C + i * 128 : C + (i + 1) * 128],
                            xo_sb[ct][:, nch * 512 : (nch + 1) * 512],
                            start=(ct == 0),
                            stop=(ct == 7),
                        )
                nc.vector.tensor_scalar_add(kh[:, :], ps[:, :], bk_sb[i][:, :])
                nc.sync.dma_start(
                    k_in[c][(i % 4) * 128 : (i % 4 + 1) * 128, :], kh[:, :]
                )
                if i % 4 == 3:
                    nc.gpsimd.collective_compute(
                        "AllGather",
                        mybir.AluOpType.bypass,
                        replica_groups=groups,
                        ins=[k_in[c].opt()],
                        outs=[k_out[c].opt()],
                    )

            # ---- V for the FULL sequence (local, natural m order; ones row
            # of x_full x bias row of wqkvT gives the V bias; col D = ones)
            for mt in range(16):
                nc.vector.memset(V_sb[mt][:, :, D : D + 1], 1.0)
            for mt in range(16):
                ps = psp.tile([128, 16, 64], F32, tag="mm", bufs=2, name="psv")
                for ct in range(9):
                    for vch in range(2):
                        nc.tensor.matmul(
                            ps[:, vch * 8 : (vch + 1) * 8, :],
                            x_sb[ct][:, mt * 128 : (mt + 1) * 128],
                            wq_sb[ct][:, 2 * C + vch * 512 : 2 * C + (vch + 1) * 512],
                            start=(ct == 0),
                            stop=(ct == 8),
                        )
                nc.vector.tensor_copy(V_sb[mt][:, :, 0:D], ps[:, :, :])

            # ---- gathered K -> SBUF
            for c in range(2):
                for r in range(2):
                    for ii in range(4):
                        i = c * 4 + ii
                        nc.sync.dma_start(
                            KT_sb[i][:, r * NQ : (r + 1) * NQ],
                            k_out[c][r, ii * 128 : (ii + 1) * 128, :],
                        )

            # ---- Q (bias fused in copy)
            for i in range(8):
                ps = psp.tile([128, NQ], F32, tag="mm", bufs=2, name="psq")
                for ct in range(8):
                    for nch in range(2):
                        nc.tensor.matmul(
                            ps[:, nch * 512 : (nch + 1) * 512],
                            wq_sb[ct][:, i * 128 : (i + 1) * 128],
                            xo_sb[ct][:, nch * 512 : (nch + 1) * 512],
                            start=(ct == 0),
                            stop=(ct == 7),
                        )
                nc.vector.tensor_scalar_add(QT_sb[i][:, :], ps[:, :], bq_sb[i][:, :])

            lp.release()
            wk = tc.alloc_tile_pool(name="attnwork", bufs=1)
            wp_sb = []
            for i in range(8):
                t = wk.tile([128, C], BF16, tag=f"wp{i}", name=f"wp{i}")
                nc.sync.dma_start(t[:, :], wprojT[i * 128 : (i + 1) * 128, :])
                wp_sb.append(t)
            pending = []

            def emit_norm():
                h, stage = pending.pop(0)
                i, poff = h // 2, (h % 2) * 64
                r = wk.tile([1, NQ], F32, tag="r", bufs=2, name="r")
                nc.vector.reciprocal(r[:, :], stage[64:65, :])
                rb = wk.tile([64, NQ], F32, tag="rb", bufs=2, name="rb")
                nc.gpsimd.partition_broadcast(rb[:, :], r[:, :])
                nc.vector.tensor_mul(
                    A_sb[i][poff : poff + 64, :], stage[0:64, :], rb[:, :]
                )

            # ---- attention (norm lags one head)
            for h in range(H):
                i, poff = h // 2, (h % 2) * 64
                pv = [
                    psp.tile([65, 512], F32, tag=f"acc{j}", bufs=2, name=f"pv{j}")
                    for j in range(2)
                ]
                for mt in range(16):
                    sp = psp.tile([128, NQ], F32, tag="mm", bufs=2, name="pss")
                    for nch in range(2):
                        nc.tensor.matmul(
                            sp[:, nch * 512 : (nch + 1) * 512],
                            KT_sb[i][poff : poff + 64, mt * 128 : (mt + 1) * 128],
                            QT_sb[i][poff : poff + 64, nch * 512 : (nch + 1) * 512],
                            start=True,
                            stop=True,
                        )
                    p = wk.tile([128, NQ], BF16, tag="p", bufs=4, name="p")
                    nc.scalar.activation(
                        p[:, :], sp[:, :],
                        mybir.ActivationFunctionType.Exp, scale=SCALE,
                    )
                    for nch in range(2):
                        nc.tensor.matmul(
                            pv[nch][:, :],
                            V_sb[mt][:, h, :],
                            p[:, nch * 512 : (nch + 1) * 512],
                            start=(mt == 0),
                            stop=(mt == 15),
                            skip_group_check=True,
                        )
                stage = wk.tile([65, NQ], BF16, tag="st", bufs=3, name="stage")
                for nch in range(2):
                    nc.vector.tensor_copy(
                        stage[:, nch * 512 : (nch + 1) * 512], pv[nch][:, :]
                    )
                pending.append((h, stage))
                if len(pending) > 1:
                    emit_norm()
            while pending:
                emit_norm()

            # ---- output projection ----
            for ot in range(8):
                pss = [
                    psp.tile([128, 512], F32, tag=f"acc{nch}", bufs=2, name="psp")
                    for nch in range(2)
                ]
                for dd in range(8):
                    for nch in range(2):
                        nc.tensor.matmul(
                            pss[nch][:, :],
                            wp_sb[dd][:, ot * 128 : (ot + 1) * 128],
                            A_sb[dd][:, nch * 512 : (nch + 1) * 512],
                            start=(dd == 0),
                            stop=(dd == 7),
                        )
                for nch in range(2):
                    y = wk.tile([128, 512], F32, tag="y", bufs=3, name="y")
                    nc.vector.tensor_scalar_add(y[:, :], pss[nch][:, :], bp_sb[ot][:, :])
                    nc.sync.dma_start(
                        yT[ot * 128 : (ot + 1) * 128, nch * 512 : (nch + 1) * 512],
                        y[:, :],
                    )
            wk.release()

    nc.compile()
    return nc


def kernel(x, w_qkv, b_qkv, w_proj, b_proj):
    global LAST_RESULTS
    bf = ml_dtypes.bfloat16
    x = np.asarray(x, np.float32)
    w_qkv = np.asarray(w_qkv, np.float32)
    b_qkv = np.asarray(b_qkv, np.float32)
    w_proj = np.asarray(w_proj, np.float32)
    b_proj = np.asarray(b_proj, np.float32)

    wqkvT = np.ascontiguousarray(
        np.vstack([w_qkv.T, b_qkv[None, :]]).astype(bf)
    )  # [1025, 3072]
    wprojT = np.ascontiguousarray(w_proj.T.astype(bf))  # [1024, 1024]
    bqk = np.ascontiguousarray(b_qkv[: 2 * C, None].astype(np.float32))  # [2048, 1]
    bproj = np.ascontiguousarray(b_proj[:, None].astype(np.float32))  # [1024, 1]

    in_maps = []
    xTb = {}
    for b in range(B):
        xTb[b] = np.ascontiguousarray(
            np.vstack([x[b].T, np.ones((1, M), np.float32)]).astype(bf)
        )
    for core in range(NCORES):
        b, half = core // 2, core % 2
        own = x[b][half * NQ : (half + 1) * NQ]  # [1024, 1024]
        in_maps.append(
            {
                "xT": xTb[b],
                "xoT": np.ascontiguousarray(own.T.astype(bf)),
                "wqkvT": wqkvT,
                "bqk": bqk,
                "wprojT": wprojT,
                "bproj": bproj,
            }
        )

    if "nc" not in _CACHE:
        _CACHE["nc"] = _build()
    nc = _CACHE["nc"]

    res = run_bass_kernel_spmd(nc, in_maps, core_ids=list(range(NCORES)))
    LAST_RESULTS = res

    out = np.empty((B, N, C), np.float32)
    for core in range(NCORES):
        b, half = core // 2, core % 2
        out[b, half * NQ : (half + 1) * NQ, :] = res.results[core]["yT"].T
    return out


if __name__ == "__main__":
    rng = np.random.default_rng(0)
    s = C ** -0.5
    ins = {
        "x": rng.standard_normal((B, N, C)).astype(np.float32),
        "w_qkv": (rng.standard_normal((3 * C, C)) * s).astype(np.float32),
        "b_qkv": (rng.standard_normal(3 * C) * 0.02).astype(np.float32),
        "w_proj": (rng.standard_normal((C, C)) * s).astype(np.float32),
        "b_proj": (rng.standard_normal(C) * 0.02).astype(np.float32),
    }
    y = kernel(**ins)
    print("out", y.shape, y.dtype, float(np.abs(y).mean()))
